# revision 15
# baseline (speedup 1.0000x reference)
"""Binarized 3x3 conv (XNOR-style): sign(conv2d(sign(x), sign(w)) + b).

Full-input contract: kernel(x=[32,256,56,56]f32, weight=[256,256,3,3]f32,
bias=[256]f32) -> [32,256,56,56]f32.

Strategy: data-parallel over batch across 8 NeuronCores (4 images/core).
Per core:
  - sign(x) encoded as +/-0.5 (exact: is_ge -> {0,1}, subtract 0.5) into a
    zero-padded 58x58 per-image layout, fp8e4 (or bf16).
  - sign(w) prepped on host as +/-1 in [c_partition, tap, (pair,) k] layout.
  - conv = 9 tap-shifted matmuls (fp8 DoubleRow, contract=256) accumulating
    into PSUM. All products are +/-0.5 with f32 accumulation, so
    psum == conv/2 exactly (conv is an even integer in [-2304, 2304]).
  - output sign = clamp(conv/2, -1, 1), exact for even integers including 0.
    One DVE tensor_scalar(min 1.0, max -1.0) per tile.
Bias is asserted zero (setup_inputs uses zeros); a nonzero bias falls back to
an exact 3-op sign path.
"""

import numpy as np

import concourse.bacc as bacc
import concourse.mybir as mybir
import concourse.tile as tile
from concourse.bass_utils import run_bass_kernel_spmd

N_CORES = 8
N_PER = 4          # images per core
C = 256            # input channels
K = 256            # output channels
H = W = 56
HP = WP = 58       # padded
XSP = HP * WP      # 3364 padded image pixels
HALF = 3376        # per-(image, pair-half) stride, padded to %16==0
RB = 8             # output rows per matmul tile
F = RB * WP        # 464 matmul free size (8 rows x 58, last 2 cols of each row garbage)
NBLK = H // RB     # 7 row blocks per image

USE_FP8 = True

_cache = {}


def _build(mode, with_bias):
    dt = mybir.dt
    xdt = dt.float8e4 if mode == "fp8" else dt.bfloat16
    nc = bacc.Bacc()
    x_d = nc.declare_dram_parameter("xs", [N_PER, C, H, W], dt.float32, isOutput=False)
    wfree = 9 * 2 * 256
    w_d = nc.declare_dram_parameter("wsgn", [128, wfree], xdt, isOutput=False)
    if with_bias:
        b_d = nc.declare_dram_parameter("bhalf", [128, 2], dt.float32, isOutput=False)
    o_d = nc.declare_dram_parameter("out", [N_PER, K, H, W], dt.float32, isOutput=True)

    with tile.TileContext(nc) as tc:
        with (
            tc.tile_pool(name="wpool", bufs=1) as wpool,
            tc.tile_pool(name="xsgn", bufs=N_PER) as xsgn_pool,
            tc.tile_pool(name="xf32", bufs=3) as xf_pool,
            tc.tile_pool(name="osb", bufs=6) as o_pool,
            tc.tile_pool(name="psum", bufs=8, space="PSUM") as p_pool,
        ):
            # Warm the PE HAM clock gate (~3.4us of activity -> 2.4 GHz)
            # while the first image is still streaming in. Results discarded;
            # source is a small zeroed tile so this depends on nothing else.
            wsrc = wpool.tile([128, 512], xdt)
            nc.gpsimd.memset(wsrc[:], 0.0)
            warm = p_pool.tile([128, F], dt.float32, tag="ps")
            for _ in range(16):
                nc.tensor.matmul(
                    warm[:], wsrc[:, 0:128], wsrc[:, 0:F],
                    start=True, stop=True,
                )
            w_sb = wpool.tile([128, wfree], xdt)
            if with_bias:
                b_sb = wpool.tile([128, 2], dt.float32)
                nc.sync.dma_start(b_sb[:], b_d[:])

            # x sign tiles: one per image, both channel halves: [128, 2*HALF]
            # free index = ci*HALF + (y*58 + x) over the padded 58x58 grid.
            # Only the borders (and the 12-elem tail pad) need zeroing; the
            # 56x56 interior is overwritten by the sign writes.
            xs_tiles = []
            xv = x_d[:].rearrange("n c h w -> n c (h w)")
            RCH = 14  # rows per input DMA/sign chunk
            for n in range(N_PER):
                xs = xsgn_pool.tile([128, 2 * HALF], xdt, tag="xsgn")
                xs_tiles.append(xs)
                for ci in range(2):
                    grid = xs[:, ci * HALF: ci * HALF + XSP].rearrange(
                        "p (h w) -> p h w", h=HP
                    )
                    nc.gpsimd.memset(grid[:, 0, :], 0.0)           # top row
                    nc.gpsimd.memset(grid[:, HP - 1, :], 0.0)      # bottom row
                    # cols 0+57 of all interior rows: adjacent pairs (r,57),(r+1,0)
                    vert = xs[:, ci * HALF + 57: ci * HALF + 57 + 57 * WP].rearrange(
                        "p (h w) -> p h w", w=WP
                    )[:, :, 0:2]
                    nc.gpsimd.memset(vert, 0.0)
                    nc.gpsimd.memset(xs[:, ci * HALF + XSP: (ci + 1) * HALF], 0.0)

            def emit_chunk(n, ch, ci):
                xs = xs_tiles[n]
                xf = xf_pool.tile([128, RCH * W], dt.float32, tag="xf32",
                                  name=f"xf_{n}_{ch}_{ci}")
                nc.sync.dma_start(
                    xf[:],
                    xv[n, ci * 128:(ci + 1) * 128,
                       ch * RCH * W: (ch + 1) * RCH * W],
                )
                dst = (
                    xs[:, ci * HALF: ci * HALF + XSP]
                    .rearrange("p (h w) -> p h w", h=HP)
                    [:, 1 + ch * RCH: 1 + (ch + 1) * RCH, 1:57]
                )
                src = xf[:].rearrange("p (h w) -> p h w", h=RCH)
                # (x>=0 -> {0,1}) - 0.5 = +/-0.5, exact
                nc.vector.tensor_scalar(
                    dst, src, 0.0, 0.5, mybir.AluOpType.is_ge,
                    mybir.AluOpType.subtract,
                )

            # image 0 first, with the (kg-split) weight DMAs slotted between
            # its first chunk pairs so everything the first matmul group
            # needs lands as early as possible
            for ch in range(H // RCH):
                emit_chunk(0, ch, 0)
                emit_chunk(0, ch, 1)
                if ch == 0:
                    nc.sync.dma_start(w_sb[:, 0: wfree // 2], w_d[:, 0: wfree // 2])
                elif ch == 1:
                    nc.sync.dma_start(w_sb[:, wfree // 2:], w_d[:, wfree // 2:])
            for n in range(1, N_PER):
                for ch in range(H // RCH):
                    emit_chunk(n, ch, 0)
                    emit_chunk(n, ch, 1)

            wv = w_sb[:].rearrange("p (g t i k) -> p g t i k", g=2, t=9, i=2)
            for n in range(N_PER):
                for kg in range(2):
                    xs = xs_tiles[n]
                    psums = [p_pool.tile([128, F], dt.float32, tag="ps", name=f"ps{kg}_{n}_{i}") for i in range(NBLK)]
                    # rb 0-2 first: those only need the first input row-chunk,
                    # so the PE can start before the whole image is signed
                    if mode == "fp8":
                        xp = xs[:].rearrange("p (i f) -> p i f", i=2)
                        for grp in (range(0, 1), range(1, 3), range(3, NBLK)):
                            for tap in range(9):
                                ty, tx = tap // 3, tap % 3
                                lhsT = wv[:, kg, tap, :, :]
                                for rb in grp:
                                    base = (rb * RB + ty) * WP + tx
                                    rhs = xp[:, :, base: base + F]
                                    nc.tensor.matmul(
                                        psums[rb][:], lhsT, rhs,
                                        start=(tap == 0), stop=(tap == 8),
                                        perf_mode=mybir.MatmulPerfMode.DoubleRow,
                                    )
                    else:
                        for grp in (range(0, 1), range(1, 3), range(3, NBLK)):
                            step = 0
                            for ci in range(2):
                                for tap in range(9):
                                    ty, tx = tap // 3, tap % 3
                                    lhsT = wv[:, kg, tap, ci, :]
                                    for rb in grp:
                                        base = ci * HALF + (rb * RB + ty) * WP + tx
                                        rhs = xs[:, base: base + F]
                                        nc.tensor.matmul(
                                            psums[rb][:], lhsT, rhs,
                                            start=(step == 0), stop=(step == 17),
                                        )
                                    step += 1
                    for rb in range(NBLK):
                        # compact the valid 8x56 (of the 8x58 psum span) so
                        # the output DMA is contiguous on both sides
                        osb = o_pool.tile([128, RB * W], dt.float32, tag="osb")
                        psv = psums[rb][:].rearrange(
                            "p (r c) -> p r c", r=RB)[:, :, 0:W]
                        ov = osb[:].rearrange("p (r c) -> p r c", r=RB)
                        if not with_bias:
                            # exact sign of even integers: clamp(v/2, -1, 1)
                            nc.vector.tensor_scalar(
                                ov, psv, 1.0, -1.0,
                                mybir.AluOpType.min, mybir.AluOpType.max,
                            )
                        else:
                            # exact sign(v + b): (v/2+b/2 > 0) - (v/2+b/2 < 0)
                            tpos = o_pool.tile([128, RB * W], dt.float32, tag="tpos")
                            tneg = o_pool.tile([128, RB * W], dt.float32, tag="tneg")
                            bcol = b_sb[:, kg: kg + 1]
                            nc.vector.tensor_scalar(
                                tpos[:].rearrange("p (r c) -> p r c", r=RB), psv,
                                bcol, 0.0,
                                mybir.AluOpType.add, mybir.AluOpType.is_gt,
                            )
                            nc.vector.tensor_scalar(
                                tneg[:].rearrange("p (r c) -> p r c", r=RB), psv,
                                bcol, 0.0,
                                mybir.AluOpType.add, mybir.AluOpType.is_lt,
                            )
                            nc.vector.tensor_tensor(
                                osb[:], tpos[:], tneg[:], mybir.AluOpType.subtract,
                            )
                        dst = o_d[n, kg * 128:(kg + 1) * 128, rb * RB: rb * RB + RB, :]
                        # stores go out via SWDGE (scalar engine) so they never
                        # queue ahead of the latency-critical input loads on
                        # the sync/HWDGE queues
                        nc.scalar.dma_start(dst, osb[:])

    nc.finalize()
    return nc


def _prep_weights(weight, mode):
    dt = mybir.dt
    xdt = dt.float8e4 if mode == "fp8" else dt.bfloat16
    sgn = np.sign(weight.astype(np.float32))
    w6 = sgn.reshape(2, 128, 2, 128, 3, 3)     # [kg, kk, i, p, ty, tx]
    arr = w6.transpose(3, 0, 4, 5, 2, 1)       # [p, kg, ty, tx, i, kk]
    arr = np.ascontiguousarray(arr).reshape(128, 9 * 2 * 256)
    return arr.astype(mybir.dt.np(xdt))


def kernel(x, weight, bias, _profile=False, _trace_kwargs=None):
    mode = "fp8" if USE_FP8 else "bf16"
    x = np.asarray(x, dtype=np.float32)
    weight = np.asarray(weight, dtype=np.float32)
    bias = np.asarray(bias, dtype=np.float32)
    with_bias = bool(np.any(bias != 0.0))

    key = (mode, with_bias)
    if key not in _cache:
        _cache[key] = _build(mode, with_bias)
    nc = _cache[key]

    wsgn = _prep_weights(weight, mode)
    in_maps = []
    for c in range(N_CORES):
        m = {
            "xs": np.ascontiguousarray(x[c * N_PER:(c + 1) * N_PER]),
            "wsgn": wsgn,
        }
        if with_bias:
            m["bhalf"] = np.ascontiguousarray(
                (bias.reshape(2, 128).T * 0.5).astype(np.float32)
            )
        in_maps.append(m)

    res = run_bass_kernel_spmd(
        nc, in_maps, core_ids=list(range(N_CORES)),
        trace=_profile, **(_trace_kwargs or {}),
    )
    out = np.concatenate([res.results[c]["out"] for c in range(N_CORES)], axis=0)
    if _profile:
        kernel.last_exec_ns = res.exec_time_ns
        kernel.last_results = res
    return out


# revision 17
# speedup vs baseline: 1.0510x; 1.0510x over previous
"""Binarized 3x3 conv (XNOR-style): sign(conv2d(sign(x), sign(w)) + b).

Full-input contract: kernel(x=[32,256,56,56]f32, weight=[256,256,3,3]f32,
bias=[256]f32) -> [32,256,56,56]f32.

Strategy: data-parallel over batch across 8 NeuronCores (4 images/core).
Per core:
  - sign(x) encoded as +/-0.5 (exact: is_ge -> {0,1}, subtract 0.5) into a
    zero-padded 58x58 per-image layout, fp8e4 (or bf16).
  - sign(w) prepped on host as +/-1 in [c_partition, tap, (pair,) k] layout.
  - conv = 9 tap-shifted matmuls (fp8 DoubleRow, contract=256) accumulating
    into PSUM. All products are +/-0.5 with f32 accumulation, so
    psum == conv/2 exactly (conv is an even integer in [-2304, 2304]).
  - output sign = clamp(conv/2, -1, 1), exact for even integers including 0.
    One DVE tensor_scalar(min 1.0, max -1.0) per tile.
Bias is asserted zero (setup_inputs uses zeros); a nonzero bias falls back to
an exact 3-op sign path.
"""

import numpy as np

import concourse.bacc as bacc
import concourse.mybir as mybir
import concourse.tile as tile
from concourse.bass_utils import run_bass_kernel_spmd

N_CORES = 8
N_PER = 4          # images per core
C = 256            # input channels
K = 256            # output channels
H = W = 56
HP = WP = 58       # padded
XSP = HP * WP      # 3364 padded image pixels
HALF = 3376        # per-(image, pair-half) stride, padded to %16==0
RB = 8             # output rows per matmul tile
F = RB * WP        # 464 matmul free size (8 rows x 58, last 2 cols of each row garbage)
NBLK = H // RB     # 7 row blocks per image

USE_FP8 = True

_cache = {}


def _build(mode, with_bias):
    dt = mybir.dt
    xdt = dt.float8e4 if mode == "fp8" else dt.bfloat16
    nc = bacc.Bacc()
    x_d = nc.declare_dram_parameter("xs", [N_PER, C, H, W], dt.float32, isOutput=False)
    wfree = 9 * 2 * 256
    w_d = nc.declare_dram_parameter("wsgn", [128, wfree], xdt, isOutput=False)
    if with_bias:
        b_d = nc.declare_dram_parameter("bhalf", [128, 2], dt.float32, isOutput=False)
    o_d = nc.declare_dram_parameter("out", [N_PER, K, H, W], dt.float32, isOutput=True)

    with tile.TileContext(nc) as tc:
        with (
            tc.tile_pool(name="wpool", bufs=1) as wpool,
            tc.tile_pool(name="xsgn", bufs=N_PER) as xsgn_pool,
            tc.tile_pool(name="xf32", bufs=3) as xf_pool,
            tc.tile_pool(name="osb", bufs=6) as o_pool,
            tc.tile_pool(name="psum", bufs=8, space="PSUM") as p_pool,
        ):
            # Warm the PE HAM clock gate (~3.4us of activity -> 2.4 GHz)
            # while the first image is still streaming in. Results discarded;
            # source is a small zeroed tile so this depends on nothing else.
            wsrc = wpool.tile([128, 512], xdt)
            nc.gpsimd.memset(wsrc[:], 0.0)
            warm = p_pool.tile([128, F], dt.float32, tag="ps")
            for _ in range(16):
                nc.tensor.matmul(
                    warm[:], wsrc[:, 0:128], wsrc[:, 0:F],
                    start=True, stop=True,
                )
            w_sb = wpool.tile([128, wfree], xdt)
            if with_bias:
                b_sb = wpool.tile([128, 2], dt.float32)
                nc.sync.dma_start(b_sb[:], b_d[:])

            # x sign tiles: one per image, both channel halves: [128, 2*HALF]
            # free index = ci*HALF + (y*58 + x) over the padded 58x58 grid.
            # (DoubleRow's rhs AP must be exactly [p, 2, N], so the pair
            # stride has to span a whole half.) Only borders + the 12-elem
            # tail pad need zeroing; the interior is sign-overwritten.
            xs_tiles = []
            xv = x_d[:].rearrange("n c h w -> n c (h w)")
            RCH = 28  # rows per input DMA/sign chunk
            for n in range(N_PER):
                xs = xsgn_pool.tile([128, 2 * HALF], xdt, tag="xsgn")
                xs_tiles.append(xs)
                for ci in range(2):
                    grid = xs[:, ci * HALF: ci * HALF + XSP].rearrange(
                        "p (h w) -> p h w", h=HP
                    )
                    nc.gpsimd.memset(grid[:, 0, :], 0.0)           # top row
                    nc.gpsimd.memset(grid[:, HP - 1, :], 0.0)      # bottom row
                    vert = xs[:, ci * HALF + 57: ci * HALF + 57 + 57 * WP].rearrange(
                        "p (h w) -> p h w", w=WP
                    )[:, :, 0:2]
                    nc.gpsimd.memset(vert, 0.0)
                    nc.gpsimd.memset(xs[:, ci * HALF + XSP: (ci + 1) * HALF], 0.0)

            def emit_chunk(n, ch, ci):
                xs = xs_tiles[n]
                xf = xf_pool.tile([128, RCH * W], dt.float32, tag="xf32",
                                  name=f"xf_{n}_{ch}_{ci}")
                nc.sync.dma_start(
                    xf[:],
                    xv[n, ci * 128:(ci + 1) * 128,
                       ch * RCH * W: (ch + 1) * RCH * W],
                )
                dst = (
                    xs[:, ci * HALF: ci * HALF + XSP]
                    .rearrange("p (h w) -> p h w", h=HP)
                    [:, 1 + ch * RCH: 1 + (ch + 1) * RCH, 1:57]
                )
                src = xf[:].rearrange("p (h w) -> p h w", h=RCH)
                # (x>=0 -> {0,1}) - 0.5 = +/-0.5, exact
                nc.vector.tensor_scalar(
                    dst, src, 0.0, 0.5, mybir.AluOpType.is_ge,
                    mybir.AluOpType.subtract,
                )

            # image 0 first, ordered by what the first matmul groups need
            # (the pair AP's byte range covers all of ci0 plus the head of
            # ci1), with the kg-split weight DMAs slotted in between
            emit_chunk(0, 0, 0)
            emit_chunk(0, 1, 0)
            emit_chunk(0, 0, 1)
            nc.sync.dma_start(w_sb[:, 0: wfree // 2], w_d[:, 0: wfree // 2])
            emit_chunk(0, 1, 1)
            nc.sync.dma_start(w_sb[:, wfree // 2:], w_d[:, wfree // 2:])
            for n in range(1, N_PER):
                for ch in range(H // RCH):
                    emit_chunk(n, ch, 0)
                    emit_chunk(n, ch, 1)

            wv = w_sb[:].rearrange("p (g t i k) -> p g t i k", g=2, t=9, i=2)
            for n in range(N_PER):
                for kg in range(2):
                    xs = xs_tiles[n]
                    psums = [p_pool.tile([128, F], dt.float32, tag="ps", name=f"ps{kg}_{n}_{i}") for i in range(NBLK)]
                    # rb 0-2 first: those only need the first input row-chunk,
                    # so the PE can start before the whole image is signed
                    if mode == "fp8":
                        xp = xs[:].rearrange("p (i f) -> p i f", i=2)
                        for grp in (range(0, 3), range(3, NBLK)):
                            for tap in range(9):
                                ty, tx = tap // 3, tap % 3
                                lhsT = wv[:, kg, tap, :, :]
                                for rb in grp:
                                    base = (rb * RB + ty) * WP + tx
                                    rhs = xp[:, :, base: base + F]
                                    nc.tensor.matmul(
                                        psums[rb][:], lhsT, rhs,
                                        start=(tap == 0), stop=(tap == 8),
                                        perf_mode=mybir.MatmulPerfMode.DoubleRow,
                                    )
                    else:
                        for grp in (range(0, 3), range(3, NBLK)):
                            step = 0
                            for ci in range(2):
                                for tap in range(9):
                                    ty, tx = tap // 3, tap % 3
                                    lhsT = wv[:, kg, tap, ci, :]
                                    for rb in grp:
                                        base = ci * HALF + (rb * RB + ty) * WP + tx
                                        rhs = xs[:, base: base + F]
                                        nc.tensor.matmul(
                                            psums[rb][:], lhsT, rhs,
                                            start=(step == 0), stop=(step == 17),
                                        )
                                    step += 1
                    for rb in range(NBLK):
                        # compact the valid 8x56 (of the 8x58 psum span) so
                        # the output DMA is contiguous on both sides
                        osb = o_pool.tile([128, RB * W], dt.float32, tag="osb")
                        psv = psums[rb][:].rearrange(
                            "p (r c) -> p r c", r=RB)[:, :, 0:W]
                        ov = osb[:].rearrange("p (r c) -> p r c", r=RB)
                        if not with_bias:
                            # exact sign of even integers: clamp(v/2, -1, 1)
                            nc.vector.tensor_scalar(
                                ov, psv, 1.0, -1.0,
                                mybir.AluOpType.min, mybir.AluOpType.max,
                            )
                        else:
                            # exact sign(v + b): (v/2+b/2 > 0) - (v/2+b/2 < 0)
                            tpos = o_pool.tile([128, RB * W], dt.float32, tag="tpos")
                            tneg = o_pool.tile([128, RB * W], dt.float32, tag="tneg")
                            bcol = b_sb[:, kg: kg + 1]
                            nc.vector.tensor_scalar(
                                tpos[:].rearrange("p (r c) -> p r c", r=RB), psv,
                                bcol, 0.0,
                                mybir.AluOpType.add, mybir.AluOpType.is_gt,
                            )
                            nc.vector.tensor_scalar(
                                tneg[:].rearrange("p (r c) -> p r c", r=RB), psv,
                                bcol, 0.0,
                                mybir.AluOpType.add, mybir.AluOpType.is_lt,
                            )
                            nc.vector.tensor_tensor(
                                osb[:], tpos[:], tneg[:], mybir.AluOpType.subtract,
                            )
                        dst = o_d[n, kg * 128:(kg + 1) * 128, rb * RB: rb * RB + RB, :]
                        # stores go out via SWDGE (scalar engine) so they never
                        # queue ahead of the latency-critical input loads on
                        # the sync/HWDGE queues
                        nc.scalar.dma_start(dst, osb[:])

    nc.finalize()
    return nc


def _prep_weights(weight, mode):
    dt = mybir.dt
    xdt = dt.float8e4 if mode == "fp8" else dt.bfloat16
    sgn = np.sign(weight.astype(np.float32))
    w6 = sgn.reshape(2, 128, 2, 128, 3, 3)     # [kg, kk, i, p, ty, tx]
    arr = w6.transpose(3, 0, 4, 5, 2, 1)       # [p, kg, ty, tx, i, kk]
    arr = np.ascontiguousarray(arr).reshape(128, 9 * 2 * 256)
    return arr.astype(mybir.dt.np(xdt))


def kernel(x, weight, bias, _profile=False, _trace_kwargs=None):
    mode = "fp8" if USE_FP8 else "bf16"
    x = np.asarray(x, dtype=np.float32)
    weight = np.asarray(weight, dtype=np.float32)
    bias = np.asarray(bias, dtype=np.float32)
    with_bias = bool(np.any(bias != 0.0))

    key = (mode, with_bias)
    if key not in _cache:
        _cache[key] = _build(mode, with_bias)
    nc = _cache[key]

    wsgn = _prep_weights(weight, mode)
    in_maps = []
    for c in range(N_CORES):
        m = {
            "xs": np.ascontiguousarray(x[c * N_PER:(c + 1) * N_PER]),
            "wsgn": wsgn,
        }
        if with_bias:
            m["bhalf"] = np.ascontiguousarray(
                (bias.reshape(2, 128).T * 0.5).astype(np.float32)
            )
        in_maps.append(m)

    res = run_bass_kernel_spmd(
        nc, in_maps, core_ids=list(range(N_CORES)),
        trace=_profile, **(_trace_kwargs or {}),
    )
    out = np.concatenate([res.results[c]["out"] for c in range(N_CORES)], axis=0)
    if _profile:
        kernel.last_exec_ns = res.exec_time_ns
        kernel.last_results = res
    return out


# revision 22
# speedup vs baseline: 1.0936x; 1.0405x over previous
"""Binarized 3x3 conv (XNOR-style): sign(conv2d(sign(x), sign(w)) + b).

Full-input contract: kernel(x=[32,256,56,56]f32, weight=[256,256,3,3]f32,
bias=[256]f32) -> [32,256,56,56]f32.

Strategy: data-parallel over batch across 8 NeuronCores (4 images/core).
Per core:
  - sign(x) encoded as +/-0.5 (exact: is_ge -> {0,1}, subtract 0.5) into
    zero-padded 58-col rows, fp8e4 (or bf16), split into two row bands per
    image (padded rows 0-33 / 32-57, 2-row halo) so every 8-row matmul span
    lives in one band and input chunks pipeline against the matmuls.
  - sign(w) prepped on host as +/-1 in [c_partition, kg, tap, pair, k] layout.
  - conv = 9 tap-shifted matmuls per 8-row block (fp8 DoubleRow, contract=256)
    accumulating into PSUM. All products are +/-0.5 with f32 accumulation, so
    psum == conv/2 exactly (conv is an even integer in [-2304, 2304]).
  - output sign = clamp(conv/2, -1, 1), exact for even integers including 0.
    One DVE tensor_scalar(min 1.0, max -1.0) per tile, then a contiguous
    store via the scalar engine's SWDGE queues (keeps HWDGE loads unblocked).
  - PE HAM warmup matmuls before the first real matmul so the 2.4 GHz
    clock gate is open from the start.
For nonzero bias the evacuation becomes (v/2+b/2>0)-(v/2+b/2<0), which
rounds identically to the reference's sign(conv+b) (binade-shift exactness).
"""

import numpy as np

import concourse.bacc as bacc
import concourse.mybir as mybir
import concourse.tile as tile
from concourse.bass_utils import run_bass_kernel_spmd

N_CORES = 8
N_PER = 4          # images per core
C = 256            # input channels
K = 256            # output channels
H = W = 56
HP = WP = 58       # padded
RB = 8             # output rows per matmul tile
F = RB * WP        # 464 matmul free size (8 rows x 58, last 2 cols of each row garbage)
NBLK = H // RB     # 7 row blocks per image

USE_FP8 = True

_cache = {}

# band split: band a = padded rows 0..33 (matmul row-blocks 0-3), band b =
# padded rows 32..57 (row-blocks 4-6); rows 32-33 are duplicated (halo) so
# every 8-row matmul span lives inside one band.  each band stores both
# channel-pair halves at a %16-padded stride, as DoubleRow requires a
# [p, 2, N] rhs access pattern.
AROWS, APAD = 34, 1984   # 34*58=1972 -> pad 1984
BROWS, BPAD = 26, 1520   # 26*58=1508 -> pad 1520
BBASE = 32               # band b's first padded row (global)
# input DMA/sign chunks: (band, orig_row0, n_rows, band_row0)
CHUNKS = [
    ("a", 0, 17, 1),     # padded rows 1..17 of band a
    ("a", 17, 16, 18),   # padded rows 18..33 of band a
    ("b", 31, 13, 0),    # band-b local rows 0..12 (halo re-fetch of rows 31-32)
    ("b", 44, 12, 13),   # band-b local rows 13..24
]


def _build(mode, with_bias):
    dt = mybir.dt
    xdt = dt.float8e4 if mode == "fp8" else dt.bfloat16
    nc = bacc.Bacc()
    x_d = nc.declare_dram_parameter("xs", [N_PER, C, H, W], dt.float32, isOutput=False)
    wfree = 9 * 2 * 256
    w_d = nc.declare_dram_parameter("wsgn", [128, wfree], xdt, isOutput=False)
    if with_bias:
        b_d = nc.declare_dram_parameter("bhalf", [128, 2], dt.float32, isOutput=False)
    o_d = nc.declare_dram_parameter("out", [N_PER, K, H, W], dt.float32, isOutput=True)

    with tile.TileContext(nc) as tc:
        with (
            tc.tile_pool(name="wpool", bufs=1) as wpool,
            tc.tile_pool(name="xsgn", bufs=2 * N_PER) as xsgn_pool,
            tc.tile_pool(name="xf32", bufs=4) as xf_pool,
            tc.tile_pool(name="osb", bufs=6) as o_pool,
            tc.tile_pool(name="psum", bufs=8, space="PSUM") as p_pool,
        ):
            # Warm the PE HAM clock gate (~3.4us of activity -> 2.4 GHz)
            # while the first image is still streaming in. Results discarded;
            # source is a small zeroed tile so this depends on nothing else.
            wsrc = wpool.tile([128, 512], xdt)
            nc.gpsimd.memset(wsrc[:], 0.0)
            warm = p_pool.tile([128, F], dt.float32, tag="ps")
            for _ in range(22):
                nc.tensor.matmul(
                    warm[:], wsrc[:, 0:128], wsrc[:, 0:F],
                    start=True, stop=True,
                )
            w_sb = wpool.tile([128, wfree], xdt)
            if with_bias:
                b_sb = wpool.tile([128, 2], dt.float32)
                nc.sync.dma_start(b_sb[:], b_d[:])

            # per-image band tiles + border zeroing (interiors get overwritten
            # by the sign writes; only borders/pads need memset)
            bands = []
            xv = x_d[:].rearrange("n c h w -> n c (h w)")
            for n in range(N_PER):
                ba = xsgn_pool.tile([128, 2 * APAD], xdt, tag="xa", name=f"xa{n}")
                bb = xsgn_pool.tile([128, 2 * BPAD], xdt, tag="xb", name=f"xb{n}")
                bands.append({"a": ba, "b": bb})
                for ci in range(2):
                    ao, bo = ci * APAD, ci * BPAD
                    # band a: top border row + left/right cols (rows 1..33,
                    # as adjacent (r,57),(r+1,0) pairs) + tail incl (33,57)
                    nc.gpsimd.memset(ba[:, ao: ao + WP], 0.0)
                    va = ba[:, ao + 57: ao + 57 + 33 * WP].rearrange(
                        "p (h w) -> p h w", w=WP)[:, :, 0:2]
                    nc.gpsimd.memset(va, 0.0)
                    nc.gpsimd.memset(ba[:, ao + 33 * WP + 57: ao + APAD], 0.0)
                    # band b: (0,0) corner + col pairs (rows 1..24 left,
                    # 0..23 right) + bottom row 25 incl (24,57) + pad
                    nc.gpsimd.memset(bb[:, bo: bo + 1], 0.0)
                    vb = bb[:, bo + 57: bo + 57 + 24 * WP].rearrange(
                        "p (h w) -> p h w", w=WP)[:, :, 0:2]
                    nc.gpsimd.memset(vb, 0.0)
                    nc.gpsimd.memset(bb[:, bo + 24 * WP + 57: bo + BPAD], 0.0)

            def emit_chunk(n, c, ci):
                band, r0, nr, br0 = CHUNKS[c]
                xt = bands[n][band]
                pad = APAD if band == "a" else BPAD
                xf = xf_pool.tile([128, nr * W], dt.float32, tag="xf32",
                                  name=f"xf_{n}_{c}_{ci}")
                nc.sync.dma_start(
                    xf[:],
                    xv[n, ci * 128:(ci + 1) * 128, r0 * W: (r0 + nr) * W],
                )
                rows = AROWS if band == "a" else BROWS
                dst = (
                    xt[:, ci * pad: ci * pad + rows * WP]
                    .rearrange("p (h w) -> p h w", w=WP)
                    [:, br0: br0 + nr, 1:57]
                )
                src = xf[:].rearrange("p (h w) -> p h w", h=nr)
                # (x>=0 -> {0,1}) - 0.5 = +/-0.5, exact
                nc.vector.tensor_scalar(
                    dst, src, 0.0, 0.5, mybir.AluOpType.is_ge,
                    mybir.AluOpType.subtract,
                )

            # image 0 ordered by what the first matmul groups need (the pair
            # AP byte range covers all of ci0's band + the head of ci1's),
            # with the kg-split weight DMAs slotted in
            nc.sync.dma_start(w_sb[:, 0: wfree // 2], w_d[:, 0: wfree // 2])
            emit_chunk(0, 0, 0)
            emit_chunk(0, 1, 0)
            nc.sync.dma_start(w_sb[:, wfree // 2:], w_d[:, wfree // 2:])
            for c, ci in ((0, 1), (1, 1), (2, 0), (3, 0), (2, 1), (3, 1)):
                emit_chunk(0, c, ci)
            for n in range(1, N_PER):
                for c in range(4):
                    emit_chunk(n, c, 0)
                    emit_chunk(n, c, 1)

            wv = w_sb[:].rearrange("p (g t i k) -> p g t i k", g=2, t=9, i=2)

            def emit_rb(n, kg, rb, split=False):
                ps = p_pool.tile([128, F], dt.float32, tag="ps",
                                 name=f"ps{kg}_{n}_{rb}")
                band = "a" if rb < 4 else "b"
                xt = bands[n][band]
                pad = APAD if band == "a" else BPAD
                rowoff = 0 if band == "a" else BBASE
                if mode == "fp8" and not split:
                    xp = xt[:].rearrange("p (i f) -> p i f", i=2)
                    for tap in range(9):
                        ty, tx = tap // 3, tap % 3
                        base = (rb * RB + ty - rowoff) * WP + tx
                        nc.tensor.matmul(
                            ps[:], wv[:, kg, tap, :, :], xp[:, :, base: base + F],
                            start=(tap == 0), stop=(tap == 8),
                            perf_mode=mybir.MatmulPerfMode.DoubleRow,
                        )
                else:
                    for step, (ci, tap) in enumerate(
                        (ci, tap) for ci in range(2) for tap in range(9)
                    ):
                        ty, tx = tap // 3, tap % 3
                        base = ci * pad + (rb * RB + ty - rowoff) * WP + tx
                        nc.tensor.matmul(
                            ps[:], wv[:, kg, tap, ci, :], xt[:, base: base + F],
                            start=(step == 0), stop=(step == 17),
                        )
                emit_evac(n, kg, rb, ps)

            def emit_evac(n, kg, rb, ps):
                # compact the valid 8x56 (of the 8x58 psum span) so the
                # output DMA is contiguous on both sides
                osb = o_pool.tile([128, RB * W], dt.float32, tag="osb",
                                  name=f"osb{kg}_{n}_{rb}")
                psv = ps[:].rearrange("p (r c) -> p r c", r=RB)[:, :, 0:W]
                ov = osb[:].rearrange("p (r c) -> p r c", r=RB)
                if not with_bias:
                    # exact sign of even integers: clamp(v/2, -1, 1)
                    nc.vector.tensor_scalar(
                        ov, psv, 1.0, -1.0,
                        mybir.AluOpType.min, mybir.AluOpType.max,
                    )
                else:
                    # exact sign(v + b): (v/2+b/2 > 0) - (v/2+b/2 < 0)
                    tpos = o_pool.tile([128, RB * W], dt.float32, tag="tpos")
                    tneg = o_pool.tile([128, RB * W], dt.float32, tag="tneg")
                    bcol = b_sb[:, kg: kg + 1]
                    nc.vector.tensor_scalar(
                        tpos[:].rearrange("p (r c) -> p r c", r=RB), psv,
                        bcol, 0.0, mybir.AluOpType.add, mybir.AluOpType.is_gt,
                    )
                    nc.vector.tensor_scalar(
                        tneg[:].rearrange("p (r c) -> p r c", r=RB), psv,
                        bcol, 0.0, mybir.AluOpType.add, mybir.AluOpType.is_lt,
                    )
                    nc.vector.tensor_tensor(
                        osb[:], tpos[:], tneg[:], mybir.AluOpType.subtract,
                    )
                dst = o_d[n, kg * 128:(kg + 1) * 128, rb * RB: rb * RB + RB, :]
                # stores go out via SWDGE (scalar engine) so they never queue
                # ahead of the latency-critical input loads on sync/HWDGE
                nc.scalar.dma_start(dst, osb[:])

            # band-a row blocks of both kg groups first, then band-b: the
            # second input band's deadline moves ~7us later, and each rb's
            # evacuation is emitted right after its taps
            for n in range(N_PER):
                for kg in range(2):
                    for rb in range(0, 4):
                        emit_rb(n, kg, rb)
                for kg in range(2):
                    for rb in range(4, NBLK):
                        emit_rb(n, kg, rb)

    nc.finalize()
    return nc

def _prep_weights(weight, mode):
    dt = mybir.dt
    xdt = dt.float8e4 if mode == "fp8" else dt.bfloat16
    sgn = np.sign(weight.astype(np.float32))
    w6 = sgn.reshape(2, 128, 2, 128, 3, 3)     # [kg, kk, i, p, ty, tx]
    arr = w6.transpose(3, 0, 4, 5, 2, 1)       # [p, kg, ty, tx, i, kk]
    arr = np.ascontiguousarray(arr).reshape(128, 9 * 2 * 256)
    return arr.astype(mybir.dt.np(xdt))


def kernel(x, weight, bias, _profile=False, _trace_kwargs=None):
    mode = "fp8" if USE_FP8 else "bf16"
    x = np.asarray(x, dtype=np.float32)
    weight = np.asarray(weight, dtype=np.float32)
    bias = np.asarray(bias, dtype=np.float32)
    assert x.shape == (N_CORES * N_PER, C, H, W), x.shape
    assert weight.shape == (K, C, 3, 3), weight.shape
    assert bias.shape == (K,), bias.shape
    with_bias = bool(np.any(bias != 0.0))

    key = (mode, with_bias)
    if key not in _cache:
        _cache[key] = _build(mode, with_bias)
    nc = _cache[key]

    wsgn = _prep_weights(weight, mode)
    in_maps = []
    for c in range(N_CORES):
        m = {
            "xs": np.ascontiguousarray(x[c * N_PER:(c + 1) * N_PER]),
            "wsgn": wsgn,
        }
        if with_bias:
            m["bhalf"] = np.ascontiguousarray(
                (bias.reshape(2, 128).T * 0.5).astype(np.float32)
            )
        in_maps.append(m)

    res = run_bass_kernel_spmd(
        nc, in_maps, core_ids=list(range(N_CORES)),
        trace=_profile, **(_trace_kwargs or {}),
    )
    out = np.concatenate([res.results[c]["out"] for c in range(N_CORES)], axis=0)
    if _profile:
        kernel.last_exec_ns = res.exec_time_ns
        kernel.last_results = res
    return out

